# revision 3
# baseline (speedup 1.0000x reference)
"""DynamicSparseMoE grouped-GEMM kernel for 8 TRN2 NeuronCores.

out[t] = tokens[t] @ weight[exp_ids[t]]   (T=8192, E=8, D=2048 -> 2048)

Strategy (expert-parallel, host-side dispatch):
  - Host sorts tokens by expert; core e owns expert e's weight and its
    routed tokens, padded to capacity C = n_full*128 (+64 if the
    remainder fits a packed half-block). Inputs cast to fp16 on host
    (PE 1 cyc/row vs fp32's 4; PSUM accumulates fp32).
  - Stationary operand is a token block x[d-block, 128t] straight from
    the resident x tiles; moving operand is a 512-wide weight slice;
    PSUM holds out[t-block, o-slice] in natural orientation.
  - Unit order: quadA (blocks 0-3 x os{0,1}, kb-major), packed-os01,
    quadB (blocks 0-3 x os{2,3}), packed-os23, pairs, then the last two
    blocks as singles. The quad os-phase halves startup weight-stream
    demand (~226 GB/s incl. x vs ~330 for a 2-block pair), the packed
    16-slot passes sit at the quad boundaries where all 8 PSUM banks
    turn over (cast slack), and singles at the end let each block's
    1 MB output stream while the next block computes, leaving only the
    final 128 KB slice in the tail.
  - The packed half-block (<=64 real tokens) runs os pairs as
    column-group-packed concurrent matmuls (tile_position from PSUM
    base partition 0/64, separate banks).
  - DMA: per-kb kicks interleaved (xA[kb], w-os0[kb], w-os1[kb]) so the
    first matmul's data is never starved by bulk transfers (the 16 DMA
    engines round-robin all in-flight transfers); later tiles (w-os23,
    xB) kick afterwards. ~22 warm-up matmuls on a memset tile bridge
    first-data latency and open the HAM clock gate.
  - PSUM->SBUF evacuation casts alternate Vector/Scalar engines (each
    cast is ~0.7 us; one engine alone falls behind the 8-bank turnover
    at unit boundaries). Output slices DMA per (block, os-slice) on the
    Scalar ring; the final slices use the idle Sync ring.
"""

import os

import numpy as np

# A previously wedged NeuronCore (NRT_EXEC_UNIT_UNRECOVERABLE) recovers on
# the next init when core reset is requested; must be set before NRT init.
os.environ.setdefault("NEURON_RT_RESET_CORES", "1")

P = 128
D = 2048
E = 8
KB = D // P  # 16 contraction blocks
NOS = 4  # 4 moving slices of 512 over the 2048 output dim
NS = D // NOS  # 512

_cache = {}


def _ensure_imports():
    try:
        import concourse.bass  # noqa: F401
    except ImportError:
        import sys

        for p in ("/opt/trn_rl_repo", "/opt/pypackages"):
            if p not in sys.path:
                sys.path.append(p)


def _np_dt(compute_dt):
    if compute_dt == "float16":
        return np.float16
    import ml_dtypes

    return ml_dtypes.bfloat16


def _build(C, packed, compute_dt="float16"):
    """Build + compile the per-core Bass program for capacity C.

    C = n_full*128 + (64 if packed else 0); requires n_full >= 6.
    """
    _ensure_imports()
    import concourse.bacc as bacc
    import concourse.mybir as mybir
    import concourse.tile as tile

    cdt = getattr(mybir.dt, compute_dt)
    n_full = (C - (64 if packed else 0)) // P
    assert n_full >= 6 and n_full * P + (64 if packed else 0) == C, (C, packed)
    NA = 4 * P  # columns covered by the quad's early x tiles

    nc = bacc.Bacc(None, target_bir_lowering=False, debug=False)
    xt_d = nc.declare_dram_parameter("xt", [D, C], cdt, isOutput=False)
    w_d = nc.declare_dram_parameter("w", [D, D], cdt, isOutput=False)
    out_d = nc.declare_dram_parameter("out", [C, D], cdt, isOutput=True)

    xt_t = xt_d.rearrange("(k p) n -> p k n", p=P)  # [128, 16, C]
    w_t = w_d.rearrange("(k p) o -> p k o", p=P)  # [128, 16, 2048]

    with tile.TileContext(nc) as tc:
        with (
            tc.tile_pool(name="wp", bufs=1) as wp,
            tc.tile_pool(name="xp", bufs=1) as xp,
            tc.tile_pool(name="op", bufs=8) as op,
            tc.tile_pool(name="pp", bufs=8, space="PSUM") as pp,
        ):
            # --- input DMA kicks (Sync ring, paced in consumption order) ---
            # Interleave per-kb so the quad's first matmuls see their data
            # within ~1.3 us of the preamble ending: the DMA engines share
            # bandwidth across ALL in-flight transfers, so small early tiles
            # must not queue behind bulk ones.
            xa = []  # [P, NA] per kb: x columns for blocks 0-3
            ws = [[None] * KB for _ in range(NOS)]  # per (os, kb) weight slices
            for kb in range(KB):
                xk = xp.tile([P, NA], cdt, tag=f"xa{kb}", name=f"xa{kb}")
                nc.sync.dma_start(xk[:], xt_t[:, kb, :NA])
                xa.append(xk)
                for osl in (0, 1):
                    wt = wp.tile([P, NS], cdt, tag=f"w{osl}_{kb}", name=f"w{osl}_{kb}")
                    nc.sync.dma_start(wt[:], w_t[:, kb, osl * NS : (osl + 1) * NS])
                    ws[osl][kb] = wt
            for osl in (2, 3):
                for kb in range(KB):
                    wt = wp.tile([P, NS], cdt, tag=f"w{osl}_{kb}", name=f"w{osl}_{kb}")
                    nc.sync.dma_start(wt[:], w_t[:, kb, osl * NS : (osl + 1) * NS])
                    ws[osl][kb] = wt
            NB = C - NA  # late x columns: blocks 4.., packed tail
            xb = []
            for kb in range(KB):
                xk = xp.tile([P, NB], cdt, tag=f"xb{kb}", name=f"xb{kb}")
                nc.sync.dma_start(xk[:], xt_t[:, kb, NA:])
                xb.append(xk)

            def lhs(kb, b):
                # stationary x block b (128 cols) for contraction block kb
                if b < 4:
                    return xa[kb][:, b * P : (b + 1) * P]
                return xb[kb][:, (b - 4) * P : (b - 3) * P]

            def lhs_packed(kb):
                return xb[kb][:, n_full * P - NA : n_full * P - NA + 64]

            # --- PE pre-warm: HAM keeps the PE clock-gated at 1.2 GHz until
            # ~3.4us of sustained activity. Dummy matmuls bridge the first-
            # data DMA latency so the gate opens ASAP.
            warm = xp.tile([P, 64], cdt, tag="warm")
            nc.vector.memset(warm[:], 0.0)
            pw = pp.tile([P, NS], mybir.dt.float32, tag="ps", name="ps_warm")
            for _ in range(22):
                nc.tensor.matmul(
                    pw[:64, :64],
                    lhsT=warm[:, :64],
                    rhs=warm[:, :64],
                    start=True,
                    stop=True,
                )

            cast_idx = [0]

            def evac(ps_ap, rows, b, osl):
                # psum -> sbuf (fp32 -> cdt cast) on alternating engines,
                # then stream the 128 KB slice out. Scalar ring carries the
                # mid-kernel output; the last slices ride the idle Sync ring.
                o_sb = op.tile([P, NS], cdt, tag="o", name=f"o_{b}_{osl}")
                eng = nc.vector if cast_idx[0] % 2 == 0 else nc.scalar
                cast_idx[0] += 1
                if eng is nc.vector:
                    eng.tensor_copy(o_sb[rows, :], ps_ap)
                else:
                    eng.copy(o_sb[rows, :], ps_ap)
                last = b >= n_full - 1 and osl >= 2
                ring = nc.sync if last else nc.scalar
                nrows = (rows.stop if rows.stop is not None else P) - (rows.start or 0)
                r0 = b * P  # packed: both halves land at rows n_full*P..+64
                ring.dma_start(
                    out_d[r0 : r0 + nrows, osl * NS : (osl + 1) * NS], o_sb[rows, :]
                )

            full = slice(0, P)

            def run_group(blocks, osls):
                # kb-major accumulation over the given (block, os) banks
                ps = {
                    (b, osl): pp.tile(
                        [P, NS], mybir.dt.float32, tag="ps", name=f"ps_{b}_{osl}"
                    )
                    for b in blocks
                    for osl in osls
                }
                for kb in range(KB):
                    for b in blocks:
                        for osl in osls:
                            nc.tensor.matmul(
                                ps[(b, osl)][:],
                                lhsT=lhs(kb, b),
                                rhs=ws[osl][kb][:],
                                start=(kb == 0),
                                stop=(kb == KB - 1),
                            )
                for b in blocks:
                    for osl in osls:
                        evac(ps[(b, osl)][:], full, b, osl)

            def run_packed(os_pair):
                # 64-wide tail tokens: the two os slices run CONCURRENTLY in
                # the PE's column-group halves (tile_position auto-derived
                # from PSUM base partition 0/64; separate banks so start=True
                # bank clears don't collide).
                ps = {
                    osl: pp.tile(
                        [P, NS], mybir.dt.float32, tag="ps", name=f"ps_pk{osl}"
                    )
                    for osl in os_pair
                }
                for kb in range(KB):
                    for osl in os_pair:
                        dst = ps[osl][:64, :] if osl % 2 == 0 else ps[osl][64:, :]
                        nc.tensor.matmul(
                            dst,
                            lhsT=lhs_packed(kb),
                            rhs=ws[osl][kb][:],
                            start=(kb == 0),
                            stop=(kb == KB - 1),
                        )
                for osl in os_pair:
                    rows = slice(0, 64) if osl % 2 == 0 else slice(64, P)
                    evac(ps[osl][rows, :], rows, n_full, osl)

            # --- unit schedule ---
            run_group([0, 1, 2, 3], (0, 1))  # quadA: rides the startup stream
            if packed:
                run_packed((0, 1))  # boundary slack; wA fully resident
            run_group([0, 1, 2, 3], (2, 3))  # quadB
            if packed:
                run_packed((2, 3))
            mids = list(range(4, n_full - 2))
            while len(mids) >= 2:
                run_group(mids[:2], (0, 1, 2, 3))
                mids = mids[2:]
            if mids:
                run_group(mids, (0, 1, 2, 3))
            run_group([n_full - 2], (0, 1, 2, 3))
            run_group([n_full - 1], (0, 1, 2, 3))
    nc.compile()
    return nc


def _get_nc(C, packed, compute_dt):
    key = (C, packed, compute_dt)
    if key not in _cache:
        _cache[key] = _build(C, packed, compute_dt)
    return _cache[key]


def kernel(tokens, weight, exp_ids, _trace=False, _compute_dt="float16"):
    _ensure_imports()
    from concourse.bass_utils import run_bass_kernel_spmd

    tokens = np.asarray(tokens)
    weight = np.asarray(weight)
    exp_ids = np.asarray(exp_ids)
    T = tokens.shape[0]

    order = np.argsort(exp_ids, kind="stable")
    counts = np.bincount(exp_ids, minlength=E)
    cap = int(counts.max())

    n_full = cap // P
    rem = cap - n_full * P
    if n_full < 6:
        n_full, rem = 6, max(rem, 0) if cap > 6 * P else 0  # tiny caps: pad up
        n_full = max(n_full, 6)
        rem = max(cap - n_full * P, 0)
    packed = 0 < rem <= 64
    if rem > 64:
        n_full += 1  # remainder too big for the packed half-block
    C = n_full * P + (64 if packed else 0)

    starts = np.zeros(E + 1, dtype=np.int64)
    np.cumsum(counts, out=starts[1:])

    npdt = _np_dt(_compute_dt)
    tokens_c = tokens.astype(npdt)
    weight_c = weight.astype(npdt)

    in_maps = []
    for e in range(E):
        idx = order[starts[e] : starts[e + 1]]
        xt = np.zeros((D, C), dtype=npdt)
        xt[:, : counts[e]] = tokens_c[idx].T
        in_maps.append({"xt": xt, "w": np.ascontiguousarray(weight_c[e])})

    nc = _get_nc(C, packed, _compute_dt)
    res = run_bass_kernel_spmd(
        nc,
        in_maps,
        core_ids=list(range(E)),
        trace=_trace,
        trace_cores=list(range(E)) if _trace else None,
    )

    out = np.empty((T, D), dtype=np.float32)
    for e in range(E):
        idx = order[starts[e] : starts[e + 1]]
        out[idx] = res.results[e]["out"][: counts[e], :].astype(np.float32)
    if _trace:
        return out, res
    return out


# revision 4
# speedup vs baseline: 1.2294x; 1.2294x over previous
"""DynamicSparseMoE grouped-GEMM kernel for 8 TRN2 NeuronCores.

out[t] = tokens[t] @ weight[exp_ids[t]]   (T=8192, E=8, D=2048 -> 2048)

Strategy (expert-parallel, host-side dispatch):
  - Host sorts tokens by expert; core e owns expert e's weight and its
    routed tokens, padded to capacity C = n_full*128 (+64 if the
    remainder fits a packed half-block). Inputs cast to fp16 on host
    (PE 1 cyc/row vs fp32's 4; PSUM accumulates fp32).
  - Stationary operand is a token block x[d-block, 128t] straight from
    the resident x tiles; moving operand is a 512-wide weight slice;
    PSUM holds out[t-block, o-slice] in natural orientation. Warm slot
    rate measured 216 ns = 512/2.4GHz + NX overhead — the floor; the
    LDWEIGHTS per 2 matmuls in the quad phases is fully hidden.
  - Unit order: quadA (blocks 0-3 x os{0,1}, kb-major), quadB (os{2,3}),
    pairs (4-os), packed-os01, singles for the last two blocks,
    packed-os23 last. The quad os-phase halves startup weight-stream
    demand (~310 GB/s incl. x vs ~330 for a 2-block pair) so matmuls
    can start ~1.6 us after the preamble instead of building a buffer;
    singles let each block's output stream while the next computes; the
    final packed half drains in ~0.7 us (two 64-row casts on parallel
    engines, out-kicks on separate rings).
  - The packed half-block (<=64 real tokens) runs os pairs as
    column-group-packed concurrent matmuls (tile_position from PSUM
    base partition 0/64, separate banks so start=True bank clears
    don't collide) — 2 matmuls per slot time.
  - DMA: the 16 DMA engines round-robin ALL in-flight transfers, so
    kick order is consumption order with small tiles first: x/wA
    interleaved per kb (first chunks split in half so the very first
    matmul's 192 KB isn't starved), then wB, then the late x columns.
    Output slices DMA per (block, os-slice): Scalar ring while Sync is
    still kicking inputs, alternating with Sync afterwards.
  - PSUM->SBUF evacuation casts alternate Vector/Scalar engines (each
    ~0.7 us; one engine alone falls behind the 8-bank turnover at unit
    boundaries). ~32 warm-up matmuls bridge first-data latency and open
    the HAM clock gate (idle gaps reset its 3.4 us activity window).
"""

import os

import numpy as np

# A previously wedged NeuronCore (NRT_EXEC_UNIT_UNRECOVERABLE) recovers on
# the next init when core reset is requested; must be set before NRT init.
os.environ.setdefault("NEURON_RT_RESET_CORES", "1")

P = 128
D = 2048
E = 8
KB = D // P  # 16 contraction blocks
NOS = 4  # 4 moving slices of 512 over the 2048 output dim
NS = D // NOS  # 512
HD = D // 2  # 1024

_cache = {}


def _ensure_imports():
    try:
        import concourse.bass  # noqa: F401
    except ImportError:
        import sys

        for p in ("/opt/trn_rl_repo", "/opt/pypackages"):
            if p not in sys.path:
                sys.path.append(p)


def _np_dt(compute_dt):
    if compute_dt == "float16":
        return np.float16
    import ml_dtypes

    return ml_dtypes.bfloat16


def _build(C, packed, compute_dt="float16"):
    """Build + compile the per-core Bass program for capacity C.

    C = n_full*128 + (64 if packed else 0); requires n_full >= 6.
    """
    _ensure_imports()
    import concourse.bacc as bacc
    import concourse.mybir as mybir
    import concourse.tile as tile

    cdt = getattr(mybir.dt, compute_dt)
    n_full = (C - (64 if packed else 0)) // P
    assert n_full >= 6 and n_full * P + (64 if packed else 0) == C, (C, packed)
    NA = 4 * P  # columns covered by the quad's early x tiles

    nc = bacc.Bacc(None, target_bir_lowering=False, debug=False)
    xt_d = nc.declare_dram_parameter("xt", [D, C], cdt, isOutput=False)
    w_d = nc.declare_dram_parameter("w", [D, D], cdt, isOutput=False)
    out_d = nc.declare_dram_parameter("out", [C, D], cdt, isOutput=True)

    xt_t = xt_d.rearrange("(k p) n -> p k n", p=P)  # [128, 16, C]
    w_t = w_d.rearrange("(k p) o -> p k o", p=P)  # [128, 16, 2048]

    with tile.TileContext(nc) as tc:
        with (
            tc.tile_pool(name="wp", bufs=1) as wp,
            tc.tile_pool(name="xp", bufs=1) as xp,
            tc.tile_pool(name="op", bufs=8) as op,
            tc.tile_pool(name="pp", bufs=8, space="PSUM") as pp,
        ):
            # --- input DMA kicks (Sync ring, consumption order) ---
            xa = []  # [P, NA] per kb: x columns for blocks 0-3
            wA = []  # [P, HD] per kb: os0+os1 weight columns
            wB = []  # [P, HD] per kb: os2+os3
            for kb in range(KB):
                xk = xp.tile([P, NA], cdt, tag=f"xa{kb}", name=f"xa{kb}")
                wk = wp.tile([P, HD], cdt, tag=f"wA{kb}", name=f"wA{kb}")
                if kb == 0:
                    # halve the first kicks so the first matmul's data
                    # (xa0 cols 0-255 + wA0 os0) isn't starved
                    nc.sync.dma_start(xk[:, : NA // 2], xt_t[:, 0, : NA // 2])
                    nc.sync.dma_start(wk[:, :NS], w_t[:, 0, :NS])
                    nc.sync.dma_start(xk[:, NA // 2 :], xt_t[:, 0, NA // 2 : NA])
                    nc.sync.dma_start(wk[:, NS:], w_t[:, 0, NS:HD])
                else:
                    nc.sync.dma_start(xk[:], xt_t[:, kb, :NA])
                    nc.sync.dma_start(wk[:], w_t[:, kb, :HD])
                xa.append(xk)
                wA.append(wk)
            for kb in range(KB):
                wk = wp.tile([P, HD], cdt, tag=f"wB{kb}", name=f"wB{kb}")
                nc.sync.dma_start(wk[:], w_t[:, kb, HD:])
                wB.append(wk)
            NB = C - NA  # late x columns: blocks 4.., packed tail
            xb = []
            for kb in range(KB):
                xk = xp.tile([P, NB], cdt, tag=f"xb{kb}", name=f"xb{kb}")
                nc.sync.dma_start(xk[:], xt_t[:, kb, NA:])
                xb.append(xk)

            def w_slice(kb, osl):
                w = wA[kb] if osl < 2 else wB[kb]
                s = (osl % 2) * NS
                return w[:, s : s + NS]

            def lhs(kb, b):
                if b < 4:
                    return xa[kb][:, b * P : (b + 1) * P]
                return xb[kb][:, (b - 4) * P : (b - 3) * P]

            def lhs_packed(kb):
                return xb[kb][:, n_full * P - NA : n_full * P - NA + 64]

            # --- PE pre-warm: HAM keeps the PE clock-gated at 1.2 GHz until
            # ~3.4us of sustained activity; dummy matmuls bridge first-data
            # DMA latency so real matmuls start as warm as possible.
            warm = xp.tile([P, 64], cdt, tag="warm")
            nc.vector.memset(warm[:], 0.0)
            pw = pp.tile([P, NS], mybir.dt.float32, tag="ps", name="ps_warm")
            for _ in range(32):
                nc.tensor.matmul(
                    pw[:64, :64],
                    lhsT=warm[:, :64],
                    rhs=warm[:, :64],
                    start=True,
                    stop=True,
                )

            cast_idx = [0]

            def evac(ps_ap, rows, b, osl, ring=None):
                # psum -> sbuf (fp32 -> cdt cast) on alternating engines,
                # then stream the 128 KB slice out.
                o_sb = op.tile([P, NS], cdt, tag="o", name=f"o_{b}_{osl}")
                use_v = cast_idx[0] % 2 == 0
                cast_idx[0] += 1
                if use_v:
                    nc.vector.tensor_copy(o_sb[rows, :], ps_ap)
                else:
                    nc.scalar.copy(o_sb[rows, :], ps_ap)
                nrows = (rows.stop if rows.stop is not None else P) - (rows.start or 0)
                r0 = b * P  # packed halves both land at rows n_full*P..+64
                ring = ring if ring is not None else nc.scalar
                ring.dma_start(
                    out_d[r0 : r0 + nrows, osl * NS : (osl + 1) * NS], o_sb[rows, :]
                )

            full = slice(0, P)

            def run_group(blocks, osls, ring=None):
                # kb-major accumulation over the given (block, os) banks
                ps = {
                    (b, osl): pp.tile(
                        [P, NS], mybir.dt.float32, tag="ps", name=f"ps_{b}_{osl}"
                    )
                    for b in blocks
                    for osl in osls
                }
                for kb in range(KB):
                    for b in blocks:
                        for osl in osls:
                            nc.tensor.matmul(
                                ps[(b, osl)][:],
                                lhsT=lhs(kb, b),
                                rhs=w_slice(kb, osl),
                                start=(kb == 0),
                                stop=(kb == KB - 1),
                            )
                for i, (b, osl) in enumerate([(b, o) for b in blocks for o in osls]):
                    r = ring[i % len(ring)] if ring else None
                    evac(ps[(b, osl)][:], full, b, osl, ring=r)

            def run_packed(os_pair, ring=None):
                # 64-wide tail tokens: the two os slices run CONCURRENTLY in
                # the PE's column-group halves (tile_position auto-derived
                # from PSUM base partition 0/64; separate banks).
                ps = {
                    osl: pp.tile(
                        [P, NS], mybir.dt.float32, tag="ps", name=f"ps_pk{osl}"
                    )
                    for osl in os_pair
                }
                for kb in range(KB):
                    for osl in os_pair:
                        dst = ps[osl][:64, :] if osl % 2 == 0 else ps[osl][64:, :]
                        nc.tensor.matmul(
                            dst,
                            lhsT=lhs_packed(kb),
                            rhs=w_slice(kb, osl),
                            start=(kb == 0),
                            stop=(kb == KB - 1),
                        )
                for i, osl in enumerate(os_pair):
                    rows = slice(0, 64) if osl % 2 == 0 else slice(64, P)
                    r = ring[i % len(ring)] if ring else None
                    evac(ps[osl][rows, :], rows, n_full, osl, ring=r)

            # --- unit schedule ---
            run_group([0, 1, 2, 3], (0, 1))  # quadA: rides the startup stream
            run_group([0, 1, 2, 3], (2, 3))  # quadB
            mids = list(range(4, n_full - 2))
            while len(mids) >= 2:
                run_group(mids[:2], (0, 1, 2, 3))
                mids = mids[2:]
            if mids:
                run_group(mids, (0, 1, 2, 3))
            both = (nc.scalar, nc.sync)  # Sync ring is free of input kicks now
            if packed:
                run_packed((0, 1), ring=both)
            run_group([n_full - 2], (0, 1, 2, 3), ring=both)
            run_group([n_full - 1], (0, 1, 2, 3), ring=both)
            if packed:
                run_packed((2, 3), ring=(nc.sync, nc.scalar))
    nc.compile()
    return nc


def _get_nc(C, packed, compute_dt):
    key = (C, packed, compute_dt)
    if key not in _cache:
        _cache[key] = _build(C, packed, compute_dt)
    return _cache[key]


def kernel(tokens, weight, exp_ids, _trace=False, _compute_dt="float16"):
    _ensure_imports()
    from concourse.bass_utils import run_bass_kernel_spmd

    tokens = np.asarray(tokens)
    weight = np.asarray(weight)
    exp_ids = np.asarray(exp_ids)
    T = tokens.shape[0]

    order = np.argsort(exp_ids, kind="stable")
    counts = np.bincount(exp_ids, minlength=E)
    cap = max(int(counts.max()), 6 * P + 1)

    n_full = cap // P
    rem = cap - n_full * P
    packed = 0 < rem <= 64
    if rem > 64:
        n_full += 1  # remainder too big for the packed half-block
    C = n_full * P + (64 if packed else 0)

    starts = np.zeros(E + 1, dtype=np.int64)
    np.cumsum(counts, out=starts[1:])

    npdt = _np_dt(_compute_dt)
    tokens_c = tokens.astype(npdt)
    weight_c = weight.astype(npdt)

    in_maps = []
    for e in range(E):
        idx = order[starts[e] : starts[e + 1]]
        xt = np.zeros((D, C), dtype=npdt)
        xt[:, : counts[e]] = tokens_c[idx].T
        in_maps.append({"xt": xt, "w": np.ascontiguousarray(weight_c[e])})

    nc = _get_nc(C, packed, _compute_dt)
    res = run_bass_kernel_spmd(
        nc,
        in_maps,
        core_ids=list(range(E)),
        trace=_trace,
        trace_cores=list(range(E)) if _trace else None,
    )

    out = np.empty((T, D), dtype=np.float32)
    for e in range(E):
        idx = order[starts[e] : starts[e + 1]]
        out[idx] = res.results[e]["out"][: counts[e], :].astype(np.float32)
    if _trace:
        return out, res
    return out


# revision 11
# speedup vs baseline: 1.2855x; 1.0456x over previous
"""DynamicSparseMoE grouped-GEMM kernel for 8 TRN2 NeuronCores.

out[t] = tokens[t] @ weight[exp_ids[t]]   (T=8192, E=8, D=2048 -> 2048)

Strategy (expert-parallel, host-side dispatch):
  - Host sorts tokens by expert; core e owns expert e's weight and its
    routed tokens, padded to capacity C = n_full*128 (+64 if the
    remainder fits a packed half-block). Inputs cast to fp16 on host
    (PE 1 cyc/row vs fp32's 4; PSUM accumulates fp32).
  - Stationary operand is a token block x[d-block, 128t] straight from
    the resident x tiles; moving operand is a 512-wide weight slice;
    PSUM holds out[t-block, o-slice] in natural orientation. Warm slot
    rate measured 216 ns = 512/2.4GHz + NX overhead — the floor; the
    LDWEIGHTS per 2 matmuls in the quad phases is fully hidden.
  - Unit order: quadA (blocks 0-3 x os{0,1}, kb-major), quadB (os{2,3}),
    pairs (4-os), packed-os01, singles for the last two blocks,
    packed-os23 last. The quad os-phase halves startup weight-stream
    demand (~310 GB/s incl. x vs ~330 for a 2-block pair) so matmuls
    can start ~1.6 us after the preamble instead of building a buffer;
    singles let each block's output stream while the next computes; the
    final packed half drains in ~0.7 us (two 64-row casts on parallel
    engines, out-kicks on separate rings).
  - The packed half-block (<=64 real tokens) runs os pairs as
    column-group-packed concurrent matmuls (tile_position from PSUM
    base partition 0/64, separate banks so start=True bank clears
    don't collide) — 2 matmuls per slot time.
  - DMA: the 16 DMA engines round-robin ALL in-flight transfers, so
    kick order is consumption order with small tiles first: x/wA
    interleaved per kb (first chunks split in half so the very first
    matmul's 192 KB isn't starved), then wB, then the late x columns.
    Output slices DMA per (block, os-slice): Scalar ring while Sync is
    still kicking inputs, alternating with Sync afterwards.
  - PSUM->SBUF evacuation casts alternate Vector/Scalar engines (each
    ~0.7 us; one engine alone falls behind the 8-bank turnover at unit
    boundaries). ~32 warm-up matmuls bridge first-data latency and open
    the HAM clock gate (idle gaps reset its 3.4 us activity window).
"""

import os

import numpy as np

# A previously wedged NeuronCore (NRT_EXEC_UNIT_UNRECOVERABLE) recovers on
# the next init when core reset is requested; must be set before NRT init.
os.environ.setdefault("NEURON_RT_RESET_CORES", "1")

P = 128
D = 2048
E = 8
KB = D // P  # 16 contraction blocks
NOS = 4  # 4 moving slices of 512 over the 2048 output dim
NS = D // NOS  # 512
HD = D // 2  # 1024

_cache = {}


def _ensure_imports():
    try:
        import concourse.bass  # noqa: F401
    except ImportError:
        import sys

        for p in ("/opt/trn_rl_repo", "/opt/pypackages"):
            if p not in sys.path:
                sys.path.append(p)


def _np_dt(compute_dt):
    if compute_dt == "float16":
        return np.float16
    import ml_dtypes

    return ml_dtypes.bfloat16


def _build(C, packed, compute_dt="float16"):
    """Build + compile the per-core Bass program for capacity C.

    C = n_full*128 + (64 if packed else 0); requires n_full >= 6.
    """
    _ensure_imports()
    import concourse.bacc as bacc
    import concourse.mybir as mybir
    import concourse.tile as tile

    cdt = getattr(mybir.dt, compute_dt)
    n_full = (C - (64 if packed else 0)) // P
    assert n_full >= 6 and n_full * P + (64 if packed else 0) == C, (C, packed)

    nc = bacc.Bacc(None, target_bir_lowering=False, debug=False)
    # x comes pre-split on the host into the quad's early columns (xA:
    # blocks 0-3) and the late columns (xB: blocks 4.., packed tail), each
    # packed kb-major so one DMA covers a PAIR of kb blocks with >=2 KB
    # per-partition rows (1 KB rows measured only ~280 GB/s vs ~340).
    NA = 4 * P
    NB = C - NA
    xA_d = nc.declare_dram_parameter("xA", [P, KB * NA], cdt, isOutput=False)
    xB_d = nc.declare_dram_parameter("xB", [P, KB * NB], cdt, isOutput=False)
    w_d = nc.declare_dram_parameter("w", [D, D], cdt, isOutput=False)
    out_d = nc.declare_dram_parameter("out", [C, D], cdt, isOutput=True)

    w_t = w_d.rearrange("(k p) o -> p k o", p=P)  # [128, 16, 2048]

    with tile.TileContext(nc) as tc:
        with (
            tc.tile_pool(name="wp", bufs=1) as wp,
            tc.tile_pool(name="xp", bufs=1) as xp,
            tc.tile_pool(name="op", bufs=10) as op,
            tc.tile_pool(name="pp", bufs=8, space="PSUM") as pp,
        ):
            # --- input DMA kicks (Sync ring, consumption order) ---
            # xa2[j] covers kb=2j,2j+1 of the quad columns; interleaved with
            # the wA stream so the startup pipeline never starves. The first
            # chunks are split in half so the very first matmul's 256 KB of
            # data isn't stuck behind bulk transfers (the 16 DMA engines
            # round-robin ALL in-flight transfers).
            xa2 = []  # [P, 2*NA] per kb-pair
            wA = []  # [P, HD] per kb: os0+os1 weight columns
            wB = []  # [P, HD] per kb: os2+os3
            for kb in range(KB):
                wA.append(wp.tile([P, HD], cdt, tag=f"wA{kb}", name=f"wA{kb}"))
                wB.append(wp.tile([P, HD], cdt, tag=f"wB{kb}", name=f"wB{kb}"))
            for j in range(KB // 2):
                xa2.append(
                    xp.tile([P, 2 * NA], cdt, tag=f"xa2_{j}", name=f"xa2_{j}")
                )
            x2 = 2 * NA
            nc.sync.dma_start(xa2[0][:, :NA], xA_d[:, :NA])
            nc.sync.dma_start(wA[0][:, :NS], w_t[:, 0, :NS])
            nc.sync.dma_start(wA[0][:, NS:], w_t[:, 0, NS:HD])
            nc.sync.dma_start(xa2[0][:, NA:], xA_d[:, NA:x2])
            nc.sync.dma_start(wA[1][:], w_t[:, 1, :HD])
            for j in range(1, KB // 2):
                nc.sync.dma_start(xa2[j][:], xA_d[:, j * x2 : (j + 1) * x2])
                nc.sync.dma_start(wA[2 * j][:], w_t[:, 2 * j, :HD])
                nc.sync.dma_start(wA[2 * j + 1][:], w_t[:, 2 * j + 1, :HD])
            for kb in range(KB):
                nc.sync.dma_start(wB[kb][:], w_t[:, kb, HD:])
            xb2 = []
            for j in range(KB // 2):
                xk = xp.tile([P, 2 * NB], cdt, tag=f"xb2_{j}", name=f"xb2_{j}")
                nc.sync.dma_start(xk[:], xB_d[:, j * 2 * NB : (j + 1) * 2 * NB])
                xb2.append(xk)

            def w_slice(kb, osl):
                w = wA[kb] if osl < 2 else wB[kb]
                s = (osl % 2) * NS
                return w[:, s : s + NS]

            def lhs(kb, b):
                if b < 4:
                    off = (kb % 2) * NA + b * P
                    return xa2[kb // 2][:, off : off + P]
                off = (kb % 2) * NB + (b - 4) * P
                return xb2[kb // 2][:, off : off + P]

            def lhs_packed(kb):
                off = (kb % 2) * NB + (n_full - 4) * P
                return xb2[kb // 2][:, off : off + 64]

            # --- PE pre-warm: HAM keeps the PE clock-gated at 1.2 GHz until
            # ~3.4us of sustained activity; dummy matmuls bridge first-data
            # DMA latency so real matmuls start as warm as possible.
            warm = xp.tile([P, 64], cdt, tag="warm")
            nc.vector.memset(warm[:], 0.0)
            pw = pp.tile([P, NS], mybir.dt.float32, tag="ps", name="ps_warm")
            for _ in range(32):
                nc.tensor.matmul(
                    pw[:64, :64],
                    lhsT=warm[:, :64],
                    rhs=warm[:, :64],
                    start=True,
                    stop=True,
                )

            cast_idx = [0]

            def evac(slices):
                # psum -> sbuf (fp32 -> cdt cast) on alternating Vector /
                # Scalar queues — casts free PSUM banks for the next unit,
                # so those queues carry NOTHING else. All casts are emitted
                # before any out-kick; the kicks ride the Sync ring, where
                # (in program order) they sit behind the input kicks and
                # drain once those finish — they never block a cast.
                kicks = []
                for ps_ap, rows, b, osl in slices:
                    o_sb = op.tile([P, NS], cdt, tag="o", name=f"o_{b}_{osl}")
                    if cast_idx[0] % 2 == 0:
                        nc.vector.tensor_copy(o_sb[rows, :], ps_ap)
                    else:
                        nc.scalar.copy(o_sb[rows, :], ps_ap)
                    cast_idx[0] += 1
                    kicks.append((o_sb, rows, b, osl))
                for o_sb, rows, b, osl in kicks:
                    nrows = (rows.stop if rows.stop is not None else P) - (
                        rows.start or 0
                    )
                    r0 = b * P  # packed halves both land at rows n_full*P..
                    nc.sync.dma_start(
                        out_d[r0 : r0 + nrows, osl * NS : (osl + 1) * NS],
                        o_sb[rows, :],
                    )

            full = slice(0, P)

            def run_group(blocks, osls):
                # kb-major accumulation over the given (block, os) banks
                ps = {
                    (b, osl): pp.tile(
                        [P, NS], mybir.dt.float32, tag="ps", name=f"ps_{b}_{osl}"
                    )
                    for b in blocks
                    for osl in osls
                }
                for kb in range(KB):
                    for b in blocks:
                        for osl in osls:
                            nc.tensor.matmul(
                                ps[(b, osl)][:],
                                lhsT=lhs(kb, b),
                                rhs=w_slice(kb, osl),
                                start=(kb == 0),
                                stop=(kb == KB - 1),
                            )
                evac(
                    [(ps[(b, o)][:], full, b, o) for b in blocks for o in osls]
                )

            def run_packed(os_pair):
                # 64-wide tail tokens: the two os slices run CONCURRENTLY in
                # the PE's column-group halves (tile_position auto-derived
                # from PSUM base partition 0/64; separate banks).
                ps = {
                    osl: pp.tile(
                        [P, NS], mybir.dt.float32, tag="ps", name=f"ps_pk{osl}"
                    )
                    for osl in os_pair
                }
                for kb in range(KB):
                    for osl in os_pair:
                        dst = ps[osl][:64, :] if osl % 2 == 0 else ps[osl][64:, :]
                        nc.tensor.matmul(
                            dst,
                            lhsT=lhs_packed(kb),
                            rhs=w_slice(kb, osl),
                            start=(kb == 0),
                            stop=(kb == KB - 1),
                        )
                evac(
                    [
                        (
                            ps[osl][
                                slice(0, 64) if osl % 2 == 0 else slice(64, P), :
                            ],
                            slice(0, 64) if osl % 2 == 0 else slice(64, P),
                            n_full,
                            osl,
                        )
                        for osl in os_pair
                    ]
                )

            # --- unit schedule ---
            run_group([0, 1, 2, 3], (0, 1))  # quadA: rides the startup stream
            run_group([0, 1, 2, 3], (2, 3))  # quadB
            mids = list(range(4, n_full - 2))
            while len(mids) >= 2:
                run_group(mids[:2], (0, 1, 2, 3))
                mids = mids[2:]
            if mids:
                run_group(mids, (0, 1, 2, 3))
            if packed:
                run_packed((0, 1))
            run_group([n_full - 2], (0, 1, 2, 3))
            run_group([n_full - 1], (0, 1, 2, 3))
            if packed:
                run_packed((2, 3))
    nc.compile()
    return nc


def _get_nc(C, packed, compute_dt):
    key = (C, packed, compute_dt)
    if key not in _cache:
        _cache[key] = _build(C, packed, compute_dt)
    return _cache[key]


def kernel(tokens, weight, exp_ids, _trace=False, _compute_dt="float16"):
    _ensure_imports()
    from concourse.bass_utils import run_bass_kernel_spmd

    tokens = np.asarray(tokens)
    weight = np.asarray(weight)
    exp_ids = np.asarray(exp_ids)
    T = tokens.shape[0]

    order = np.argsort(exp_ids, kind="stable")
    counts = np.bincount(exp_ids, minlength=E)
    cap = max(int(counts.max()), 6 * P + 1)

    n_full = cap // P
    rem = cap - n_full * P
    packed = 0 < rem <= 64
    if rem > 64:
        n_full += 1  # remainder too big for the packed half-block
    C = n_full * P + (64 if packed else 0)

    starts = np.zeros(E + 1, dtype=np.int64)
    np.cumsum(counts, out=starts[1:])

    npdt = _np_dt(_compute_dt)
    tokens_c = tokens.astype(npdt)
    weight_c = weight.astype(npdt)

    NA = 4 * P
    NB = C - NA
    in_maps = []
    for e in range(E):
        idx = order[starts[e] : starts[e + 1]]
        xt = np.zeros((D, C), dtype=npdt)
        xt[:, : counts[e]] = tokens_c[idx].T
        # kb-major packs so the device reads >=2 KB per-partition rows:
        # xA[p, kb*NA + t] = xt[kb*128+p, t],  t < NA (quad blocks 0-3)
        # xB[p, kb*NB + t] = xt[kb*128+p, NA+t]     (late blocks + tail)
        x3 = xt.reshape(KB, P, C)
        xA = np.ascontiguousarray(
            x3[:, :, :NA].transpose(1, 0, 2).reshape(P, KB * NA)
        )
        xB = np.ascontiguousarray(
            x3[:, :, NA:].transpose(1, 0, 2).reshape(P, KB * NB)
        )
        in_maps.append({"xA": xA, "xB": xB, "w": np.ascontiguousarray(weight_c[e])})

    nc = _get_nc(C, packed, _compute_dt)
    res = run_bass_kernel_spmd(
        nc,
        in_maps,
        core_ids=list(range(E)),
        trace=_trace,
        trace_cores=list(range(E)) if _trace else None,
    )

    out = np.empty((T, D), dtype=np.float32)
    for e in range(E):
        idx = order[starts[e] : starts[e + 1]]
        out[idx] = res.results[e]["out"][: counts[e], :].astype(np.float32)
    if _trace:
        return out, res
    return out
